# revision 15
# baseline (speedup 1.0000x reference)
"""DCNv2 (deformable conv v2) TRN2 Bass kernel — data-parallel over batch on
8 NeuronCores (one batch image per core).

v2: transpose-mode dma_gather. Per core: image reformatted into a packed
channels-last fp16 DRAM table where row p holds the 2x2 pixel patch
[p, p+1, p+64, p+65] (4*256 fp16 = 2 KB). One transpose-gather per
(kernel-position k, 1024-site raster block b) pulls whole patches and lands
them CHANNEL-MAJOR: G[c, 2q+cc, i] = img[corner q, ch cc*128+c] for stream
site i = 1024*b + i — no PE transposes in the main loop. Bilinear corner
weights alpha (fp16, computed compactly on DVE) round-trip through a DRAM
staging buffer into per-(k,b) [1, 4*1024] rows, are broadcast across
partitions by K=1 ones-matmuls into PSUM quarters, and evicted to fp16
SBUF by the ACT engine. The DVE then forms cols = sum_q alpha_q * G_q with
4 mults + 3 adds per block (all fp16, alpha re-read across the two 128-ch
halves via a 0-stride broadcast AP). fp16 matmuls accumulate po[oh] over
(k, ch) into PSUM (bias seeded via a K=1 matmul); ACT evicts po to a
raster outS, one f32 DMA out.

The SWDGE idx tile holds stream position i at (partition i%16, col i//16),
replicated across the 8 Q7 core groups; for raster streams the content at
(p, col=cc*2+c2) is the patch-base row of site (32*b+cc)*32 + c2*16 + p,
built from a PE transpose of the compact index tile.
"""
import sys
sys.path.insert(0, '/opt/trn_rl_repo')

import numpy as np
import concourse.bass as bass
import concourse.bacc as bacc
import concourse.mybir as mybir
import concourse.tile as tile
from concourse.bass_types import AP
from concourse.bass_utils import run_bass_kernel_spmd
from concourse import library_config, masks

dt = mybir.dt
Alu = mybir.AluOpType
ActFn = mybir.ActivationFunctionType

B = 8
C, H, W = 256, 64, 64
HW = H * W
O = 256
KK = 9
PADR = 128                 # channels-last scratch pad rows each side
SROWS = PADR + HW + PADR
PADR2 = 66                 # packed-patch scratch top pad rows
S2ROWS = PADR2 + HW + 2    # packed rows; max row idx used is PADR2+HW-1
NB = 4                     # site blocks (1024 sites each)
NIDX = 1024
F16 = dt.float16
F32 = dt.float32
I16 = dt.int16
I32 = dt.int32

_NC_CACHE = {}

# --- queue-aware DMASW semaphore-lane assignment -------------------------
# Tile rotates Pool-engine DMA completion sems over 8 DMASW lanes in
# scheduled order; a lane gets locked to the SWDGE queue that first uses
# it. With 2 SWDGE queues the rotation must keep lane parity == queue, so
# partition the lanes: queue q uses lanes {q, q+2, q+4, q+6}.
import concourse.tile_sem_assignment as _tsa

_orig_assign_tick = _tsa.TileClockTick._assign_tick


def _assign_tick_qaware(self, inst):
    qn = getattr(inst, "queue_num", None)
    if (isinstance(inst, _tsa.DMAInst)
            and inst.engine == mybir.EngineType.Pool and qn is not None):
        if not hasattr(self, "_q_rot"):
            self._q_rot = {}
        nq = 2
        r = self._q_rot.get(qn, 0)
        self._q_rot[qn] = (r + 1) % (self.swdge_sem_count // nq)
        self.next_sw_dma_idx = (qn + nq * r) % self.swdge_sem_count
    return _orig_assign_tick(self, inst)


_tsa.TileClockTick._assign_tick = _assign_tick_qaware


def build_nc(num_swdge_queues=2, dbg=False):
    nc = bacc.Bacc("TRN2", target_bir_lowering=False, debug=True,
                   num_swdge_queues=num_swdge_queues)
    if dbg:
        dgp = nc.dram_tensor("dgp", [128, 8, NIDX], F16, kind="ExternalOutput")
        dabc = nc.dram_tensor("dabc", [128, 4, NIDX], F16,
                              kind="ExternalOutput")
        dcols = nc.dram_tensor("dcols", [128, 2, NIDX], F16,
                               kind="ExternalOutput")
        dtall = nc.dram_tensor("dtall", [128, NB, KK, 64], I16,
                               kind="ExternalOutput")
    inp = nc.dram_tensor("input", [C, HW], F32, kind="ExternalInput")
    off = nc.dram_tensor("offset", [2 * KK, HW], F32, kind="ExternalInput")
    msk = nc.dram_tensor("mask", [KK, HW], F32, kind="ExternalInput")
    wgt = nc.dram_tensor("weight", [O, C * KK], F32, kind="ExternalInput")
    bia = nc.dram_tensor("bias", [O], F32, kind="ExternalInput")
    out = nc.dram_tensor("out", [O, HW], F32, kind="ExternalOutput")
    scratch = nc.dram_tensor("scratch", [SROWS, C], F16)
    scr2 = nc.dram_tensor("scr2", [S2ROWS, 4 * C], F16)
    alphad = nc.dram_tensor("alphad", [KK, NB, 4, NIDX], F16)

    with tile.TileContext(nc) as tc:
        with tc.tile_pool(name="const", bufs=1) as pc, \
             tc.tile_pool(name="psum_mm", bufs=1, space="PSUM") as pmm:

            nc.gpsimd.load_library(library_config.mlp)

            ident = pc.tile([128, 128], F16)
            masks.make_identity(nc, ident[:])
            identf = pc.tile([128, 128], F32)
            masks.make_identity(nc, identf[:])

            wt = pc.tile([128, KK, 2, 2, 128], F16)   # [cc, k, ch, oh, o]
            tall = pc.tile([128, NB, KK, 64], I16)    # gather idx tiles
            ones = pc.tile([1, 1024], F16)
            nc.gpsimd.memset(ones[:], 1.0)
            biash = pc.tile([1, O], F16)
            zpad = pc.tile([128, C], F16)
            nc.gpsimd.memset(zpad[:], 0.0)
            nc.sync.dma_start(scratch[0:PADR, :], zpad[:])
            nc.sync.dma_start(scratch[PADR + HW:SROWS, :], zpad[:])

            with tc.tile_pool(name="prep", bufs=1) as pp, \
                 tc.tile_pool(name="prep2", bufs=2) as pp2, \
                 tc.tile_pool(name="psum_tp", bufs=2, space="PSUM") as ptp:
                # --- image -> channels-last scratch ---
                imgf = pp.tile([128, 2, HW], F32)
                nc.sync.dma_start(
                    imgf[:], inp[:].rearrange("(ch p) f -> p ch f", p=128))
                imgh = pp.tile([128, 2, HW], F16)
                nc.vector.tensor_copy(imgh[:, 0], imgf[:, 0])
                nc.vector.tensor_copy(imgh[:, 1], imgf[:, 1])
                for pb2 in range(16):
                    tp2 = ptp.tile([128, 1024], F16, tag="tp")
                    for t in range(2):
                        for ch in range(2):
                            nc.tensor.transpose(
                                tp2[:, (2 * t + ch) * 128:(2 * t + ch + 1) * 128],
                                imgh[:, ch, (2 * pb2 + t) * 128:(2 * pb2 + t + 1) * 128],
                                ident[:])
                    st = pp2.tile([128, 2, 2, 128], F16, tag="st")
                    nc.scalar.activation(
                        st[:].rearrange("p t ch f -> p (t ch f)"),
                        tp2[:, 0:512], ActFn.Copy)
                    nc.sync.dma_start(
                        scratch[PADR + pb2 * 256:PADR + (pb2 + 1) * 256, :]
                        .rearrange("(t p) (ch cc) -> p t ch cc", p=128, ch=2),
                        st[:])

                # --- packed 2x2 patch scratch: row p = pixels p,p+1,p+64,p+65
                s2v = scr2[:].rearrange("r (q c) -> r q c", q=4)
                for q, dlt in enumerate([0, 1, W, W + 1]):
                    src0 = PADR - PADR2 + dlt
                    nc.sync.dma_start(s2v[:, q, :],
                                      scratch[src0:src0 + S2ROWS, :])

                # --- weights -> lhsT tiles ---
                wldf = pp.tile([128, 2, C * KK], F32)
                nc.sync.dma_start(
                    wldf[:], wgt[:].rearrange("(oh p) ck -> p oh ck", p=128))
                wldh = pp.tile([128, 2, C * KK], F16)
                nc.vector.tensor_copy(wldh[:, 0], wldf[:, 0])
                nc.vector.tensor_copy(wldh[:, 1], wldf[:, 1])
                for k in range(KK):
                    tpw = ptp.tile([128, 1024], F16, tag="tp")
                    wview = wldh[:].rearrange("p oh (c k) -> p oh c k", k=KK)
                    for ch in range(2):
                        for oh in range(2):
                            nc.tensor.transpose(
                                tpw[:, (ch * 2 + oh) * 128:(ch * 2 + oh + 1) * 128],
                                wview[:, oh, ch * 128:(ch + 1) * 128, k], ident[:])
                    nc.scalar.activation(
                        wt[:, k].rearrange("p ch oh f -> p (ch oh f)"),
                        tpw[:, 0:512], ActFn.Copy)

                biasf = pp.tile([1, O], F32)
                nc.sync.dma_start(biasf[:], bia[:].rearrange("(a f) -> a f", a=1))
                nc.vector.tensor_copy(biash[:], biasf[:])

                # --- offsets/mask -> compact alphas + gather indices ---
                # site s = 32*p + j lives at (partition p, col j)
                offT = pp.tile([128, 2 * KK, 32], F32)
                nc.sync.dma_start(
                    offT[:], off[:].rearrange("c (p j) -> p c j", j=32))
                mT = pp.tile([128, KK, 32], F32)
                nc.sync.dma_start(
                    mT[:], msk[:].rearrange("c (p j) -> p c j", j=32))

                # h (row) index for partition p: h = p // 2; w = 32*(p%2) + j
                ia32 = pp.tile([128, 1], I32)
                nc.gpsimd.iota(ia32[:], [[1, 1]], base=0, channel_multiplier=1)
                iaf = pp.tile([128, 1], F32)
                nc.vector.tensor_copy(iaf[:], ia32[:])
                hraw = pp.tile([128, 1], F32)
                nc.vector.tensor_scalar(hraw[:], iaf[:], 0.5, None, Alu.mult)
                hi = pp.tile([128, 1], I32)
                nc.vector.tensor_copy(hi[:], hraw[:])
                hf = pp.tile([128, 1], F32)
                nc.vector.tensor_copy(hf[:], hi[:])
                hgt = pp.tile([128, 1], F32)
                nc.vector.tensor_tensor(hgt[:], hf[:], hraw[:], Alu.is_gt)
                h_ap = pp.tile([128, 1], F32)
                nc.vector.tensor_tensor(h_ap[:], hf[:], hgt[:], Alu.subtract)
                am32 = pp.tile([128, 1], F32)
                nc.vector.scalar_tensor_tensor(am32[:], h_ap[:], -2.0, iaf[:],
                                               Alu.mult, Alu.add)
                nc.vector.tensor_scalar(am32[:], am32[:], 32.0, None, Alu.mult)
                jio = pp.tile([128, 32], I32)
                nc.gpsimd.iota(jio[:], [[1, 32]], base=0, channel_multiplier=0)
                jf = pp.tile([128, 32], F32)
                nc.vector.tensor_copy(jf[:], jio[:])
                w32 = pp.tile([128, 32], F32)
                nc.vector.tensor_scalar(w32[:], jf[:], am32[:], None, Alu.add)

                py = pp.tile([128, KK, 32], F32)
                px = pp.tile([128, KK, 32], F32)
                for k in range(KK):
                    ki, kj = k // 3, k % 3
                    nc.vector.tensor_scalar(py[:, k], offT[:, 2 * k], h_ap[:],
                                            float(ki - 1), Alu.add, Alu.add)
                    nc.vector.tensor_scalar(px[:, k], offT[:, 2 * k + 1],
                                            float(kj - 1), None, Alu.add)
                    nc.vector.tensor_tensor(px[:, k], px[:, k], w32[:], Alu.add)

                def floorf(src, flo, frac, nm):
                    ti = pp.tile([128, KK, 32], I32, tag=f"fl_i_{nm}", name=f"fi_{nm}")
                    nc.vector.tensor_copy(ti[:], src)
                    tf = pp.tile([128, KK, 32], F32, tag=f"fl_f_{nm}", name=f"ff_{nm}")
                    nc.vector.tensor_copy(tf[:], ti[:])
                    gt = pp.tile([128, KK, 32], F32, tag=f"fl_g_{nm}", name=f"fg_{nm}")
                    nc.vector.tensor_tensor(gt[:], tf[:], src, Alu.is_gt)
                    nc.vector.tensor_tensor(flo, tf[:], gt[:], Alu.subtract)
                    nc.vector.tensor_tensor(frac, src, flo, Alu.subtract)

                y0 = pp.tile([128, KK, 32], F32)
                ly = pp.tile([128, KK, 32], F32)
                floorf(py[:], y0[:], ly[:], "y")
                x0 = pp.tile([128, KK, 32], F32)
                lx = pp.tile([128, KK, 32], F32)
                floorf(px[:], x0[:], lx[:], "x")

                def cmp2(src, lo, hi_, nm):
                    t1 = pp.tile([128, KK, 32], F32, tag=f"c1_{nm}", name=f"t1_{nm}")
                    nc.vector.tensor_scalar(t1[:], src, lo, None, Alu.is_ge)
                    t2 = pp.tile([128, KK, 32], F32, tag=f"c2_{nm}", name=f"t2_{nm}")
                    nc.vector.tensor_scalar(t2[:], src, hi_, None, Alu.is_le)
                    nc.vector.tensor_tensor(t1[:], t1[:], t2[:], Alu.mult)
                    return t1

                vy0 = cmp2(y0[:], 0.0, 63.0, "vy0")
                vy1 = cmp2(y0[:], -1.0, 62.0, "vy1")
                vx0 = cmp2(x0[:], 0.0, 63.0, "vx0")
                vx1 = cmp2(x0[:], -1.0, 62.0, "vx1")

                oly = pp.tile([128, KK, 32], F32)
                nc.vector.tensor_scalar(oly[:], ly[:], -1.0, 1.0, Alu.mult, Alu.add)
                olx = pp.tile([128, KK, 32], F32)
                nc.vector.tensor_scalar(olx[:], lx[:], -1.0, 1.0, Alu.mult, Alu.add)

                ry0 = pp.tile([128, KK, 32], F32)
                nc.vector.tensor_tensor(ry0[:], oly[:], mT[:], Alu.mult)
                nc.vector.tensor_tensor(ry0[:], ry0[:], vy0[:], Alu.mult)
                ry1 = pp.tile([128, KK, 32], F32)
                nc.vector.tensor_tensor(ry1[:], ly[:], mT[:], Alu.mult)
                nc.vector.tensor_tensor(ry1[:], ry1[:], vy1[:], Alu.mult)
                cx0 = pp.tile([128, KK, 32], F32)
                nc.vector.tensor_tensor(cx0[:], olx[:], vx0[:], Alu.mult)
                cx1 = pp.tile([128, KK, 32], F32)
                nc.vector.tensor_tensor(cx1[:], lx[:], vx1[:], Alu.mult)

                al = pp.tile([128, 4, KK, 32], F32)   # corners 00,01,10,11
                nc.vector.tensor_tensor(al[:, 0], ry0[:], cx0[:], Alu.mult)
                nc.vector.tensor_tensor(al[:, 1], ry0[:], cx1[:], Alu.mult)
                nc.vector.tensor_tensor(al[:, 2], ry1[:], cx0[:], Alu.mult)
                nc.vector.tensor_tensor(al[:, 3], ry1[:], cx1[:], Alu.mult)
                alc = pp.tile([128, 4, KK, 32], F16)
                nc.vector.tensor_copy(alc[:], al[:])
                # stage alphas to DRAM as [k, b, q, i] (i = in-block site);
                # site s = 32*p + j = 1024*b + i, so p = 32*b + p2,
                # i = 32*p2 + j. Per-partition runs are 64 B.
                for b in range(NB):
                    for q in range(4):
                        nc.sync.dma_start(
                            alphad[:, b, q].rearrange(
                                "k (p2 j) -> p2 k j", j=32),
                            alc[32 * b:32 * b + 32, q, :, :])

                # patch-base table row: i0t = clamp(y0)*W + clamp(x0) + PADR2
                cy0 = pp.tile([128, KK, 32], F32)
                nc.vector.tensor_scalar(cy0[:], y0[:], -1.0, 63.0, Alu.max, Alu.min)
                cxc = pp.tile([128, KK, 32], F32)
                nc.vector.tensor_scalar(cxc[:], x0[:], -1.0, 63.0, Alu.max, Alu.min)
                cxp0 = pp.tile([128, KK, 32], F32)
                nc.vector.tensor_scalar(cxp0[:], cxc[:], float(PADR2), None, Alu.add)

                # --- idx tiles: stream i of block b = site 1024*b + i at
                # (partition i%16, col i//16); col = cc*2 + c2 where
                # site = (32*b + cc)*32 + c2*16 + (i%16).
                # tall[p, b, k, cc*2+c2] = i0t[32b+cc, k, 16*c2+p]
                # i0t = cy0*W + cxp0 is formed AFTER fp16 PE transposes of
                # its small-range ingredients (cy0 <= 63, cxp0 <= 129 are
                # fp16-exact; i0t itself can reach 4161 and is not).
                cyh = pp.tile([128, KK, 32], F16)
                nc.vector.tensor_copy(cyh[:], cy0[:])
                cxh = pp.tile([128, KK, 32], F16)
                nc.vector.tensor_copy(cxh[:], cxp0[:])
                iTs = pp.tile([32, KK, 128], F32)
                for k in range(KK):
                    cyxT = ptp.tile([32, 2, 128], F16, tag="cyxT",
                                    name=f"cyxT_{k}")
                    nc.tensor.transpose(cyxT[:, 0, :], cyh[:, k, :], ident[:])
                    nc.tensor.transpose(cyxT[:, 1, :], cxh[:, k, :], ident[:])
                    cyxS = pp2.tile([32, 2, 128], F16, tag="cyxS",
                                    name=f"cyxS_{k}")
                    nc.scalar.activation(
                        cyxS[:].rearrange("p a c -> p (a c)"),
                        cyxT[:].rearrange("p a c -> p (a c)"), ActFn.Copy)
                    nc.vector.scalar_tensor_tensor(iTs[:, k, :], cyxS[:, 0, :],
                                                   float(W), cyxS[:, 1, :],
                                                   Alu.mult, Alu.add)
                iTs2 = pp.tile([16, KK, 128], F32)
                nc.sync.dma_start(iTs2[:], iTs[16:32])
                tview = tall[0:16, :, :, :].rearrange(
                    "p b k (cc c2) -> p c2 b k cc", c2=2)
                for b in range(NB):
                    nc.vector.tensor_copy(
                        tview[:, 0, b], iTs[0:16, :, 32 * b:32 * b + 32])
                    nc.vector.tensor_copy(
                        tview[:, 1, b], iTs2[0:16, :, 32 * b:32 * b + 32])
                for g in range(1, 8):
                    nc.sync.dma_start(tall[16 * g:16 * g + 16], tall[0:16])

            # ---------------- main loop ----------------
            with tc.tile_pool(name="mainp", bufs=1) as pm, \
                 tc.tile_pool(name="gpool", bufs=3) as pg, \
                 tc.tile_pool(name="arowp", bufs=3) as par, \
                 tc.tile_pool(name="abcp", bufs=2) as pab, \
                 tc.tile_pool(name="tp_", bufs=2) as ptt, \
                 tc.tile_pool(name="colsp", bufs=3) as pcl, \
                 tc.tile_pool(name="psum_a", bufs=2, space="PSUM") as pal:
                outS = pm.tile([128, 2, HW], F32)
                for b in range(NB):
                    po = [pmm.tile([128, NIDX], F32, tag=f"mo{oh}",
                                   name=f"po{oh}_{b}")
                          for oh in range(2)]
                    for oh in range(2):
                        for n2 in range(2):
                            nc.tensor.matmul(
                                po[oh][:, n2 * 512:(n2 + 1) * 512],
                                biash[0:1, oh * 128:(oh + 1) * 128],
                                ones[0:1, 0:512], start=True, stop=False)
                    for k in range(KK):
                        # gather: G[c, 2q+cc, i] = img[corner q, cc*128+c]
                        gp = pg.tile([128, 8, NIDX], F16, tag="g",
                                     name=f"g_{b}_{k}")
                        nc.gpsimd.dma_gather(gp[:], scr2[:], tall[:, b, k, :],
                                             NIDX, NIDX, 4 * C,
                                             transpose=True,
                                             single_packet=False,
                                             queue_num=0)
                        # alpha row [1, 4*1024] from DRAM staging
                        arow = par.tile([1, 4, NIDX], F16, tag="ar",
                                        name=f"ar_{b}_{k}")
                        nc.sync.dma_start(
                            arow[:].rearrange("one q i -> one (q i)"),
                            alphad[:].rearrange("k nb q i -> k (nb q i)")
                            [k:k + 1, b * 4 * NIDX:(b + 1) * 4 * NIDX])
                        # broadcast across partitions: K=1 ones matmuls into
                        # PSUM quarters, ACT evicts to fp16
                        abc = pab.tile([128, 4, NIDX], F16, tag="abc",
                                       name=f"abc_{b}_{k}")
                        for q in range(4):
                            aps = pal.tile([128, NIDX], F32, tag="aps",
                                           name=f"aps_{b}_{k}_{q}")
                            for n2 in range(2):
                                nc.tensor.matmul(
                                    aps[:, n2 * 512:(n2 + 1) * 512],
                                    ones[0:1, 0:128],
                                    arow[0:1, q, n2 * 512:(n2 + 1) * 512],
                                    start=True, stop=True)
                            nc.scalar.activation(abc[:, q, :], aps[:],
                                                 ActFn.Copy)
                        # cols = sum_q alpha_q * G_q   (fp16 DVE)
                        ta = ptt.tile([128, 2, NIDX], F16, tag="ta",
                                      name=f"ta_{b}_{k}")
                        tb = ptt.tile([128, 2, NIDX], F16, tag="tb",
                                      name=f"tb_{b}_{k}")
                        cols = pcl.tile([128, 2, NIDX], F16, tag="cols",
                                        name=f"cols_{b}_{k}")

                        def abq(q):
                            # alpha_q broadcast over the 2 channel halves
                            return abc[:, q, :].unsqueeze(1).broadcast_to(
                                (128, 2, NIDX))

                        nc.vector.tensor_tensor(ta[:], gp[:, 0:2, :], abq(0),
                                                Alu.mult)
                        nc.vector.tensor_tensor(tb[:], gp[:, 2:4, :], abq(1),
                                                Alu.mult)
                        nc.vector.tensor_tensor(ta[:], ta[:], tb[:], Alu.add)
                        nc.vector.tensor_tensor(tb[:], gp[:, 4:6, :], abq(2),
                                                Alu.mult)
                        nc.vector.tensor_tensor(cols[:], gp[:, 6:8, :], abq(3),
                                                Alu.mult)
                        nc.vector.tensor_tensor(tb[:], tb[:], cols[:], Alu.add)
                        nc.vector.tensor_tensor(cols[:], ta[:], tb[:], Alu.add)

                        if dbg and b == 0 and k == 0:
                            nc.sync.dma_start(dgp[:], gp[:])
                            nc.sync.dma_start(dabc[:], abc[:])
                            nc.sync.dma_start(dcols[:], cols[:])
                            nc.sync.dma_start(dtall[:], tall[:])

                        for oh in range(2):
                            for ch in range(2):
                                for n2 in range(2):
                                    nc.tensor.matmul(
                                        po[oh][:, n2 * 512:(n2 + 1) * 512],
                                        wt[:, k, ch, oh, :],
                                        cols[:, ch, n2 * 512:(n2 + 1) * 512],
                                        start=False,
                                        stop=(k == KK - 1 and ch == 1))
                    for oh in range(2):
                        nc.scalar.activation(
                            outS[:, oh, b * NIDX:(b + 1) * NIDX], po[oh][:],
                            ActFn.Copy)
                nc.sync.dma_start(
                    out[:].rearrange("(oh p) f -> p oh f", p=128), outS[:])
    nc.compile()
    return nc


def _get_nc():
    if "nc" not in _NC_CACHE:
        _NC_CACHE["nc"] = build_nc()
    return _NC_CACHE["nc"]


def kernel(**inputs):
    inp = np.ascontiguousarray(np.asarray(inputs["input"], dtype=np.float32))
    off = np.ascontiguousarray(np.asarray(inputs["offset"], dtype=np.float32))
    msk = np.ascontiguousarray(np.asarray(inputs["mask"], dtype=np.float32))
    wgt = np.ascontiguousarray(np.asarray(inputs["weight"], dtype=np.float32))
    bia = np.ascontiguousarray(np.asarray(inputs["bias"], dtype=np.float32))
    assert inp.shape == (B, C, H, W)

    wflat = wgt.reshape(O, C * KK)
    in_maps = []
    for b in range(B):
        in_maps.append({
            "input": inp[b].reshape(C, HW),
            "offset": off[b].reshape(2 * KK, HW),
            "mask": msk[b].reshape(KK, HW),
            "weight": wflat,
            "bias": bia,
        })
    nc = _get_nc()
    res = run_bass_kernel_spmd(nc, in_maps, list(range(B)))
    out = np.stack([res.results[b]["out"].reshape(O, H, W) for b in range(B)])
    return out.astype(np.float32)


if __name__ == "__main__":
    rng = np.random.default_rng(0)
    ins = {
        "input": rng.standard_normal((B, C, H, W)).astype(np.float32),
        "offset": rng.standard_normal((B, 2 * KK, H, W)).astype(np.float32),
        "mask": rng.random((B, KK, H, W)).astype(np.float32),
        "weight": rng.uniform(-1 / 48, 1 / 48, (O, C, 3, 3)).astype(np.float32),
        "bias": np.zeros((O,), np.float32),
    }
    o = kernel(**ins)
    print("kernel ran, out shape", o.shape, "finite:", np.isfinite(o).all())
